# revision 1
# baseline (speedup 1.0000x reference)
"""Multi-head attention (B=2, S=2048, D=1024, H=16) on 8 Trainium2 cores.

Sharding: tensor-parallel over heads — each core owns 2 heads (a 128-feature
slice) for both batches.  Per core:
  - QKV projections for its feature slice (full tokens), transposed layout
  - causal attention for its 4 (batch, head) pairs with block-skipping
  - partial output projection (contraction over its 128 features)
Host: transposes/prepares inputs, sums the 8 partial outputs, adds bo.

On-chip pipeline: inputs/weights arrive in IN_DT (bf16 default), q/k
projections are stored float32r and the score matmuls run float32r (full PE
rate at moving-dim >= 256, near-fp32 precision); the v path and attention
probabilities are bf16 (PV runs bf16 at full rate).  v is transposed to
token-major with small PE transposes.  The causal mask is applied with
GPSIMD affine_select on the exp output — masked tiles' PV matmuls are
deferred to the end of each block so the in-order PE never waits on the
(slow) mask op, and fully-masked column prefixes of diagonal tiles are
skipped in scores/exp/PV.  Cross-phase work (next-batch projections,
finished-block output projections) is interleaved into the attention
instruction stream with DMA-rate pacing.
"""

import os

import numpy as np
import ml_dtypes

D_MODEL = 1024
NUM_HEADS = 16
DEPTH = 64
BATCH = 2
SEQ = 2048
NTOK = BATCH * SEQ  # 4096
N_CORES = 8
FW = 128  # features per core (2 heads x 64)
P = 128
SB = 512  # s-block width
N_SB = SEQ // SB  # 4 s-blocks per batch
N_TT = SEQ // P  # 16 t-tiles per batch
N_NB = NTOK // SB  # 8 n-blocks over all tokens
N_CT = D_MODEL // P  # 8 contraction tiles

# bf16 inputs halve DMA traffic; fp32 inputs maximize accuracy.
IN_BF16 = os.environ.get("MHA_IN_BF16", "1") == "1"

LAST_RESULTS = None  # BassKernelResults from the most recent kernel() call
LAST_EXEC_WALL = None  # wall seconds of the run_bass_kernel_spmd call


def _mask_structure(mask_np):
    """Classify each (t-tile, s-block) of the [S, S] mask (1.0 = disallowed).

    Returns (kind, mix_idx, patterns): kind[i][j] in
    {"skip", "plain", "affine", "mixed"}; for "affine", mix_idx[i][j] is the
    offset c of keep = (s >= c + t); for "mixed" it indexes into patterns
    (list of [P, SB] keep-masks).  mask rows = query s, cols = key t;
    scoresT is [t, s] so we transpose.
    """
    maskT = np.ascontiguousarray(mask_np.reshape(SEQ, SEQ).T)
    kind = [[None] * N_SB for _ in range(N_TT)]
    mix_idx = [[None] * N_SB for _ in range(N_TT)]
    patterns = []
    pat_key = {}
    s_idx = np.arange(SB)[None, :]
    t_idx = np.arange(P)[:, None]
    for i in range(N_TT):
        for j in range(N_SB):
            sub = maskT[i * P : (i + 1) * P, j * SB : (j + 1) * SB]
            if np.all(sub >= 0.5):
                kind[i][j] = "skip"
                continue
            if np.all(sub < 0.5):
                kind[i][j] = "plain"
                continue
            keep = (sub < 0.5).astype(np.float32)
            first_one = np.argmax(keep, axis=1)
            c = int(first_one[0])
            if np.array_equal(keep, (s_idx >= c + t_idx).astype(np.float32)):
                kind[i][j] = "affine"
                mix_idx[i][j] = c
                continue
            kind[i][j] = "mixed"
            key = keep.tobytes()
            if key not in pat_key:
                pat_key[key] = len(patterns)
                patterns.append(keep)
            mix_idx[i][j] = pat_key[key]
    return kind, mix_idx, patterns


def _build_nc(kind, mix_idx, n_patterns, in_bf16, has_bias):
    import concourse.tile as tile
    import concourse.mybir as mybir
    from concourse import bacc

    dt = mybir.dt
    # walrus requires every producer feeding an fp32r matmul to emit
    # fp32r-typed (rounded) values, so fp32-mode inputs and the q/k/ao/wo
    # activations are declared float32r outright (same bytes as fp32).
    IN_DT = dt.bfloat16 if in_bf16 else dt.float32r

    nc = bacc.Bacc(None, target_bir_lowering=False)

    qT = nc.dram_tensor("qT", [D_MODEL, NTOK], IN_DT, kind="ExternalInput")
    kT = nc.dram_tensor("kT", [D_MODEL, NTOK], IN_DT, kind="ExternalInput")
    vT = nc.dram_tensor("vT", [D_MODEL, NTOK], IN_DT, kind="ExternalInput")
    wq = nc.dram_tensor("wqT", [D_MODEL, FW], IN_DT, kind="ExternalInput")
    wk = nc.dram_tensor("wkT", [D_MODEL, FW], IN_DT, kind="ExternalInput")
    wv = nc.dram_tensor("wvT", [D_MODEL, FW], IN_DT, kind="ExternalInput")
    wo = nc.dram_tensor("woT", [FW, D_MODEL], dt.float32r, kind="ExternalInput")
    maskt = None
    if n_patterns:
        maskt = nc.dram_tensor(
            "maskt", [n_patterns, P, SB], dt.bfloat16, kind="ExternalInput"
        )
    bq = bk = bv = None
    if has_bias:
        bq = nc.dram_tensor("bq", [FW, 1], dt.float32, kind="ExternalInput")
        bk = nc.dram_tensor("bk", [FW, 1], dt.float32, kind="ExternalInput")
        bv = nc.dram_tensor("bv", [FW, 1], dt.float32, kind="ExternalInput")
    identT = nc.dram_tensor("identT", [P, 64], dt.bfloat16, kind="ExternalInput")
    outT = nc.dram_tensor("outT", [D_MODEL, NTOK], dt.bfloat16, kind="ExternalOutput")

    with tile.TileContext(nc) as tc:
        with (
            tc.tile_pool(name="const", bufs=1) as const,
            tc.tile_pool(name="big", bufs=1) as big,
            tc.tile_pool(name="stageA", bufs=6) as stA,
            tc.tile_pool(name="expp", bufs=6) as expp,
            tc.tile_pool(name="linp", bufs=2) as linp,
            tc.tile_pool(name="outst", bufs=6) as outst,
            # PSUM: sc 2x2 banks + pv 2x1 + shared proj/outproj 2 = 8 banks
            tc.tile_pool(name="mmps", bufs=2, space="PSUM") as mmps,
            tc.tile_pool(name="scps", bufs=2, space="PSUM") as scps,
            tc.tile_pool(name="pvps", bufs=1, space="PSUM") as pvps,
        ):
            # ---- constants ----
            wq_sb = const.tile([P, N_CT, FW], IN_DT)
            wk_sb = const.tile([P, N_CT, FW], IN_DT)
            wv_sb = const.tile([P, N_CT, FW], IN_DT)
            nc.sync.dma_start(wq_sb, wq.rearrange("(ct p) f -> p ct f", p=P))
            nc.sync.dma_start(wk_sb, wk.rearrange("(ct p) f -> p ct f", p=P))
            nc.sync.dma_start(wv_sb, wv.rearrange("(ct p) f -> p ct f", p=P))
            wo_sb = const.tile([P, N_CT, P], dt.float32r)
            nc.sync.dma_start(wo_sb, wo.rearrange("f (dt p) -> f dt p", p=P))
            mask_sb = None
            if n_patterns:
                mask_sb = const.tile([P, n_patterns, SB], dt.bfloat16)
                nc.sync.dma_start(mask_sb, maskt.rearrange("m p s -> p m s"))
            # [128, 64] = eye(64) stacked twice (bf16): PE-transpose identity;
            # the h=1 slice needs base partition 64.
            ident = const.tile([P, 64], dt.bfloat16)
            nc.sync.dma_start(ident, identT[:, :])
            bias_sb = {}
            if has_bias:
                for name, t in (("q", bq), ("k", bk), ("v", bv)):
                    bias_sb[name] = const.tile([P, 1], dt.float32)
                    nc.sync.dma_start(bias_sb[name], t[:, :])

            # ---- persistent activations ----
            qh_sb = big.tile([P, NTOK], dt.float32r)
            kh_sb = big.tile([P, NTOK], dt.float32r)
            ao_sb = big.tile([P, NTOK], dt.float32r)
            vhT_sb = big.tile([P, NTOK], dt.bfloat16)
            # per (b, h): [t', t-tile, 128] bf16; one 64-col half holds v
            # (written by PE transposes), the other is 1.0 so the PV
            # matmul also produces the softmax row-sums l:
            #   h0: lhsT = [v | 1] -> psum [data(0:64); l(64:128)]
            #   h1: lhsT = [1 | v] -> psum [l(0:64); data(64:128)]
            vh_sb = [
                big.tile([P, N_TT, P], dt.bfloat16, name=f"vh{pair}")
                for pair in range(4)
            ]
            for pair in range(4):
                h = pair % 2
                ones_sl = slice(64, 128) if h == 0 else slice(0, 64)
                nc.vector.memset(vh_sb[pair][:, :, ones_sl], 1.0)

            def proj_copyback(dst_ap, ps, bias_tile):
                if bias_tile is not None:
                    nc.vector.tensor_tensor(
                        dst_ap, ps, bias_tile.to_broadcast(ps.shape),
                        mybir.AluOpType.add,
                    )
                else:
                    nc.vector.tensor_copy(dst_ap, ps)

            def project_nb(src, w_sb, dst, bn, nb):
                """Project one 512-token block of one of q/k/vT."""
                st = stA.tile([P, N_CT, SB], IN_DT, tag="st", name="st")
                nc.sync.dma_start(
                    st,
                    src[:, nb * SB : (nb + 1) * SB].rearrange(
                        "(ct p) n -> p ct n", p=P
                    ),
                )
                ps = mmps.tile([P, SB], dt.float32, tag="ps", name="ps")
                for ct in range(N_CT):
                    nc.tensor.matmul(
                        ps,
                        lhsT=w_sb[:, ct, :],
                        rhs=st[:, ct, :],
                        start=(ct == 0),
                        stop=(ct == N_CT - 1),
                    )
                proj_copyback(
                    dst[:, nb * SB : (nb + 1) * SB],
                    ps,
                    bias_sb.get(bn) if has_bias else None,
                )

            def transpose_vh(b, h, m):
                """One 512-token chunk of vhT [64, SB] -> vh natural
                [128, 4, 64] via PE transposes (the XBAR DMA transpose
                serializes against every in-flight DMA on mode switch)."""
                pair = 2 * b + h
                data_sl = slice(0, 64) if h == 0 else slice(64, 128)
                hsl = slice(h * 64, h * 64 + 64)
                for tt in range(4 * m, 4 * m + 4):
                    tok0 = b * SEQ + tt * P
                    pst = mmps.tile([P, 64], dt.bfloat16, tag="ps", name="tp")
                    nc.tensor.transpose(
                        pst, vhT_sb[hsl, tok0 : tok0 + P], ident[hsl, :]
                    )
                    nc.vector.tensor_copy(vh_sb[pair][:, tt, data_sl], pst)

            def flush_filler_upto(b, j):
                """Emit all queued units that attention block (b, j) depends
                on (its own batch's projections/transposes up to block j)."""
                while filler and (
                    filler[0][2] is None or filler[0][2] <= (b, j)
                ):
                    unit, cost, _ = filler.popleft()
                    unit()

            def attention_block(b, j):
                flush_filler_upto(b, j)
                ilist = [i for i in range(N_TT) if kind[i][j] != "skip"]
                assert ilist, "fully-masked s-block unsupported"
                pv = [
                    pvps.tile([P, SB], dt.float32, tag=f"pv{h}", name=f"pv{h}")
                    for h in range(2)
                ]
                ssl = slice(b * SEQ + j * SB, b * SEQ + (j + 1) * SB)
                n_pv = len(ilist)
                pv_emitted = 0
                deferred = []

                def emit_pv(i, e, s0):
                    nonlocal pv_emitted
                    for h in range(2):
                        nc.tensor.matmul(
                            pv[h][:, s0:],
                            lhsT=vh_sb[2 * b + h][:, i, :],
                            rhs=e[:, h, s0:],
                            start=(pv_emitted == 0),
                            stop=(pv_emitted == n_pv - 1),
                        )
                    pv_emitted += 1

                # an affine tile with offset c has columns s' < c fully
                # masked: skip them in scores/exp/PV. Only safe if the
                # first-emitted PV (which clears the psum accumulation
                # group) covers the whole block.
                can_trim = any(
                    kind[i][j] == "plain"
                    or (kind[i][j] == "affine" and mix_idx[i][j] == 0)
                    for i in ilist
                )

                def trim(i):
                    if can_trim and kind[i][j] == "affine":
                        return min(mix_idx[i][j], SB)
                    return 0

                # masked tiles first so their (slow, GPSIMD) mask ops get the
                # whole block to complete (their PVs are deferred to the end);
                # within them, smallest trim first so the first-emitted PV
                # covers the whole block (it clears the psum accumulation).
                ilist.sort(key=lambda i: (kind[i][j] == "plain", trim(i)))
                for i in ilist:
                    s0 = trim(i)
                    # fp32r score matmuls drop to 1/4 rate below 256 moving
                    s0_sc = min(s0, SB - 256)
                    tsl = slice(b * SEQ + i * P, b * SEQ + (i + 1) * P)
                    sc = scps.tile([P, 2, SB], dt.float32, tag="sc", name="sc")
                    for h in range(2):
                        hs = slice(h * 64, h * 64 + 64)
                        nc.tensor.matmul(
                            sc[:, h, s0_sc:],
                            lhsT=kh_sb[hs, tsl],
                            rhs=qh_sb[hs, ssl][:, s0_sc:],
                            start=True,
                            stop=True,
                        )
                    e = expp.tile([P, 2, SB], dt.bfloat16, tag="e", name="e")
                    nc.scalar.activation(
                        e[:, :, s0:],
                        sc[:, :, s0:],
                        mybir.ActivationFunctionType.Exp,
                        scale=1.0 / float(np.sqrt(DEPTH)),
                    )
                    if kind[i][j] == "affine":
                        # zero e[t', h, s'] where s' < c + t' (GPSIMD)
                        nc.gpsimd.affine_select(
                            out=e[:, :, s0:],
                            in_=e[:, :, s0:],
                            pattern=[[0, 2], [1, SB - s0]],
                            compare_op=mybir.AluOpType.is_ge,
                            fill=0.0,
                            base=s0 - mix_idx[i][j],
                            channel_multiplier=-1,
                        )
                        deferred.append((i, e, s0))
                    elif kind[i][j] == "mixed":
                        u = mix_idx[i][j]
                        nc.vector.tensor_tensor(
                            e,
                            e,
                            mask_sb[:, u, None, :].to_broadcast(e.shape),
                            mybir.AluOpType.mult,
                        )
                        deferred.append((i, e, 0))
                    else:
                        emit_pv(i, e, 0)
                    # interleave cross-phase work, paced so a DMA-heavy
                    # projection unit (~2.9us of DMA) gets ~3 score
                    # iterations (~1us each) of headroom. Single ordered
                    # queue: emission order must respect dataflow.
                    budget[0] += 1
                    while filler and budget[0] >= filler[0][1]:
                        unit, cost, _ = filler.popleft()
                        budget[0] -= cost
                        unit()
                for i, e, s0 in deferred:
                    emit_pv(i, e, s0)
                # normalize: l must land on the data's partitions; engines
                # can't cross partitions, so bounce through an SBUF DMA (on
                # the gpsimd queue to keep the sync sequencer free).
                ltmp = linp.tile([P, SB], dt.float32, tag="ltmp", name="ltmp")
                nc.vector.tensor_copy(ltmp[64:128, :], pv[0][64:128, :])
                nc.vector.tensor_copy(ltmp[0:64, :], pv[1][0:64, :])
                lin = linp.tile([P, SB], dt.float32, tag="lin", name="lin")
                nc.gpsimd.dma_start(lin[0:64, :], ltmp[64:128, :])
                nc.gpsimd.dma_start(lin[64:128, :], ltmp[0:64, :])
                nc.vector.reciprocal(lin, lin)
                nc.vector.tensor_tensor(
                    ao_sb[0:64, ssl], pv[0][0:64, :], lin[0:64, :],
                    mybir.AluOpType.mult,
                )
                nc.vector.tensor_tensor(
                    ao_sb[64:128, ssl], pv[1][64:128, :], lin[64:128, :],
                    mybir.AluOpType.mult,
                )

            def outproj_nb(nb, dgroup):
                """Output projection for n-block nb, d-tiles [2*dgroup, +2)."""
                nsl = slice(nb * SB, (nb + 1) * SB)
                ost = outst.tile([P, 2, SB], dt.bfloat16, tag="ost", name="ost")
                for u in range(2):
                    dtile = 2 * dgroup + u
                    ps = mmps.tile([P, SB], dt.float32, tag="ps", name="po")
                    nc.tensor.matmul(
                        ps,
                        lhsT=wo_sb[:, dtile, :],
                        rhs=ao_sb[:, nsl],
                        start=True,
                        stop=True,
                    )
                    nc.vector.tensor_copy(ost[:, u, :], ps)
                nc.sync.dma_start(
                    outT[:, nsl]
                    .rearrange("(ct p) n -> p ct n", p=P)[:, 2 * dgroup : 2 * dgroup + 2, :],
                    ost,
                )

            def proj_units(b, m):
                nb = b * (N_NB // 2) + m
                return [
                    lambda: project_nb(kT, wk_sb, kh_sb, "k", nb),
                    lambda: project_nb(qT, wq_sb, qh_sb, "q", nb),
                    lambda: project_nb(vT, wv_sb, vhT_sb, "v", nb),
                ]

            # driver: only block (0,0)'s projections run serially; later
            # projections, v transposes and output projections of finished
            # blocks are interleaved into the attention stream.
            from collections import deque

            filler = deque()
            budget = [0]

            def push_block_units(b, m):
                for u in proj_units(b, m):
                    filler.append((u, 3, (b, m)))
                filler.append((lambda x=m, y=b: transpose_vh(y, 0, x), 1, (b, m)))
                filler.append((lambda x=m, y=b: transpose_vh(y, 1, x), 1, (b, m)))

            for u in proj_units(0, 0):
                u()
            transpose_vh(0, 0, 0)
            transpose_vh(0, 1, 0)
            for m in range(1, N_SB):
                push_block_units(0, m)
            for m in range(N_SB):
                push_block_units(1, m)
            for b in range(BATCH):
                for j in range(N_SB):
                    attention_block(b, j)
                    for dg in range(N_CT // 2):
                        filler.append(
                            (
                                lambda x=b * (N_NB // 2) + j, g=dg: outproj_nb(x, g),
                                1,
                                None,
                            )
                        )
            while filler:
                filler.popleft()[0]()

    nc.compile()
    return nc


def kernel(v, k, q, mask, Wq, bq, Wk, bk, Wv, bv, Wo, bo, trace=False):
    global LAST_RESULTS
    from concourse.bass_utils import run_bass_kernel_spmd

    in_np = ml_dtypes.bfloat16 if IN_BF16 else np.float32

    def prep_T(x):  # [B, S, D] -> [D, NTOK] in input dtype
        return np.ascontiguousarray(
            np.asarray(x, dtype=np.float32).reshape(NTOK, D_MODEL).T
        ).astype(in_np)

    qT = prep_T(q)
    kT = prep_T(k)
    vT = prep_T(v)

    kind, mix_idx, patterns = _mask_structure(np.asarray(mask, dtype=np.float32))
    maskt = (
        np.ascontiguousarray(np.stack(patterns)).astype(ml_dtypes.bfloat16)
        if patterns
        else None
    )

    has_bias = bool(
        np.any(np.asarray(bq)) or np.any(np.asarray(bk)) or np.any(np.asarray(bv))
    )

    nc = _build_nc(kind, mix_idx, len(patterns), IN_BF16, has_bias)

    in_maps = []
    for core in range(N_CORES):
        fsl = slice(core * FW, (core + 1) * FW)
        m = {
            "qT": qT,
            "kT": kT,
            "vT": vT,
            "wqT": np.ascontiguousarray(np.asarray(Wq)[fsl].T).astype(in_np),
            "wkT": np.ascontiguousarray(np.asarray(Wk)[fsl].T).astype(in_np),
            "wvT": np.ascontiguousarray(np.asarray(Wv)[fsl].T).astype(in_np),
            "woT": np.ascontiguousarray(np.asarray(Wo)[:, fsl].T).astype(np.float32),
            "identT": np.concatenate(
                [np.eye(64, dtype=np.float32)] * 2, axis=0
            ).astype(ml_dtypes.bfloat16),
        }
        if maskt is not None:
            m["maskt"] = maskt
        if has_bias:
            m["bq"] = np.asarray(bq, np.float32)[fsl].reshape(FW, 1)
            m["bk"] = np.asarray(bk, np.float32)[fsl].reshape(FW, 1)
            m["bv"] = np.asarray(bv, np.float32)[fsl].reshape(FW, 1)
        in_maps.append(m)

    import time as _time

    global LAST_EXEC_WALL
    _t0 = _time.time()
    res = run_bass_kernel_spmd(
        nc, in_maps, core_ids=list(range(N_CORES)), trace=trace
    )
    LAST_EXEC_WALL = _time.time() - _t0
    LAST_RESULTS = res

    acc = np.zeros((D_MODEL, NTOK), dtype=np.float32)
    for r in res.results:
        acc += r["outT"].astype(np.float32)
    acc += np.asarray(bo, np.float32)[:, None]
    return np.ascontiguousarray(acc.T).reshape(BATCH, SEQ, D_MODEL)



# revision 74
# speedup vs baseline: 1.3345x; 1.3345x over previous
"""Multi-head attention (B=2, S=2048, D=1024, H=16) on 8 Trainium2 cores.

Sharding: 2-way data parallel over batch x 4-way tensor parallel over heads.
Each core owns one batch and 4 heads (a 256-feature slice).  Per core:
  - QKV projections for its feature slice over its batch's 2048 tokens
  - causal attention for its 4 heads with block-skipping
  - partial output projection (contraction over its 256 features)
Host: transposes/prepares inputs, sums the 4 partial outputs per batch, adds
bo.

On-chip pipeline (vs the previous head-only-TP kernel):
  - v is projected DIRECTLY into token-major layout (lhsT = the v input
    tile, moving dim = features), eliminating the PE transposes and their
    vector-engine copybacks.
  - the output projection DMAs fp32 psum straight to DRAM (no psum->SBUF
    copy, no bf16 cast); the host sums fp32 partials.
  - the causal mask affine_select only touches the 128-column window that
    straddles the diagonal (everything right of it is fully kept).
  - softmax row-sums come for free from ones-columns in the PV lhsT; the
    cross-partition bounce uses GPSIMD partition_broadcast (or an SBUF DMA
    fallback, MHA_NORM=dma).
  - q/k projections are stored float32r; score matmuls run float32r at
    full PE rate (moving dim >= 256).  v/probs are bf16.
Cross-phase work (next-block projections, finished-block output
projections) is interleaved into the attention stream with budget pacing.
"""

import os

import numpy as np
import ml_dtypes

D_MODEL = 1024
NUM_HEADS = 16
DEPTH = 64
BATCH = 2
SEQ = 2048
N_CORES = 8
H_LOC = 4  # heads per core
FW = 256  # features per core (4 heads x 64)
P = 128
SB = 512  # s-block width
NTOK_LOC = SEQ  # tokens per core (one batch)
N_SB = SEQ // SB  # 4 s-blocks
N_TT = SEQ // P  # 16 t-tiles
N_NB = NTOK_LOC // SB  # 4 n-blocks
N_CT = D_MODEL // P  # 8 contraction tiles
N_FT = FW // P  # 2 feature tiles
N_HP = H_LOC // 2  # 2 head-pairs

NORM_MODE = os.environ.get("MHA_NORM", "mmb")  # mmb | dma
# fp8 3-term projections: inputs/weights are split hi+lo into fp8e4 on the
# host; each 128-deep contraction tile needs 3 of the 4 cross products
# (x8*w8, x8*wr, xr*w8 -- the dropped xr*wr term is O(quant_err^2)), and
# DoubleRow packs 2 products per matmul at 0.5 cycles/row: 12 instructions
# replace 8 at 0.75x the PE time with ~2x BETTER accuracy than bf16 inputs.
# Weights are pre-scaled by W_SCALE so their residuals stay out of the
# fp8 subnormal range; the scale is folded into the exp scale and Wo.
FP8 = os.environ.get("MHA_FP8", "1") == "1"
W_SCALE = 32.0
# filler pacing costs (in score-tile-iteration credits)
C_DMA = int(os.environ.get("MHA_C_DMA", "1"))
C_QK = int(os.environ.get("MHA_C_QK", "2"))
C_V = int(os.environ.get("MHA_C_V", "2"))
C_OP = int(os.environ.get("MHA_C_OP", "3"))

LAST_RESULTS = None
LAST_EXEC_WALL = None


def _mask_structure(mask_np):
    """Classify each (t-tile, s-block) of the [S, S] mask (1.0 = disallowed).

    Returns (kind, mix_idx, patterns): kind[i][j] in
    {"skip", "plain", "affine", "mixed"}; for "affine", mix_idx[i][j] is the
    offset c of keep = (s >= c + t); for "mixed" it indexes into patterns
    (list of [P, SB] keep-masks).  mask rows = query s, cols = key t;
    scoresT is [t, s] so we transpose.
    """
    maskT = np.ascontiguousarray(mask_np.reshape(SEQ, SEQ).T)
    kind = [[None] * N_SB for _ in range(N_TT)]
    mix_idx = [[None] * N_SB for _ in range(N_TT)]
    patterns = []
    pat_key = {}
    s_idx = np.arange(SB)[None, :]
    t_idx = np.arange(P)[:, None]
    for i in range(N_TT):
        for j in range(N_SB):
            sub = maskT[i * P : (i + 1) * P, j * SB : (j + 1) * SB]
            if np.all(sub >= 0.5):
                kind[i][j] = "skip"
                continue
            if np.all(sub < 0.5):
                kind[i][j] = "plain"
                continue
            keep = (sub < 0.5).astype(np.float32)
            first_one = np.argmax(keep, axis=1)
            c = int(first_one[0])
            if np.array_equal(keep, (s_idx >= c + t_idx).astype(np.float32)):
                kind[i][j] = "affine"
                mix_idx[i][j] = c
                continue
            kind[i][j] = "mixed"
            key = keep.tobytes()
            if key not in pat_key:
                pat_key[key] = len(patterns)
                patterns.append(keep)
            mix_idx[i][j] = pat_key[key]
    return kind, mix_idx, patterns


def _build_nc(kind, mix_idx, n_patterns, has_bias):
    import concourse.tile as tile
    import concourse.mybir as mybir
    from concourse import bacc

    dt = mybir.dt

    nc = bacc.Bacc(None, target_bir_lowering=False)

    if FP8:
        # inputs ship as (hi, lo) fp8e4 pairs interleaved per 512-token
        # block (so DMA slices stay <= 3 dims); weights interleave
        # (lo, hi) at full width.  The orders make the DoubleRow slot APs
        # of all three product groups regular-stride slices.
        IN_DT = dt.float8e4
        in_shape = [D_MODEL, 2 * NTOK_LOC]
        w_shape = [D_MODEL, 2 * FW]
    else:
        IN_DT = dt.bfloat16
        in_shape = [D_MODEL, NTOK_LOC]
        w_shape = [D_MODEL, FW]
    qT = nc.dram_tensor("qT", in_shape, IN_DT, kind="ExternalInput")
    kT = nc.dram_tensor("kT", in_shape, IN_DT, kind="ExternalInput")
    vT = nc.dram_tensor("vT", in_shape, IN_DT, kind="ExternalInput")
    wq = nc.dram_tensor("wqT", w_shape, IN_DT, kind="ExternalInput")
    wk = nc.dram_tensor("wkT", w_shape, IN_DT, kind="ExternalInput")
    wv = nc.dram_tensor("wvT", w_shape, IN_DT, kind="ExternalInput")
    wo = nc.dram_tensor("woT", [FW, D_MODEL], dt.float32r, kind="ExternalInput")
    maskt = None
    if n_patterns:
        maskt = nc.dram_tensor(
            "maskt", [n_patterns, P, SB], dt.bfloat16, kind="ExternalInput"
        )
    bq = bk = bv = None
    if has_bias:
        bq = nc.dram_tensor("bq", [FW, 1], dt.float32, kind="ExternalInput")
        bk = nc.dram_tensor("bk", [FW, 1], dt.float32, kind="ExternalInput")
        bv = nc.dram_tensor("bv", [FW, 1], dt.float32, kind="ExternalInput")
    # cols [NTOK_LOC, NTOK_LOC+SB) hold the kt=1 partial of the last
    # n-block (its outproj is split by feature-half so the first half can
    # run during the final attention block); host sums the two partials.
    outT = nc.dram_tensor(
        "outT", [D_MODEL, NTOK_LOC + SB], dt.bfloat16, kind="ExternalOutput"
    )
    outT_r = outT.rearrange("(dt p) n -> p dt n", p=P)

    with tile.TileContext(nc) as tc:
        with (
            tc.tile_pool(name="const", bufs=1) as const,
            tc.tile_pool(name="big", bufs=1) as big,
            tc.tile_pool(name="stageA", bufs=2) as stA,
            tc.tile_pool(name="expp", bufs=6) as expp,
            tc.tile_pool(name="linp", bufs=2) as linp,
            # PSUM: mm 2x1 banks + sc 2x2 banks + pv 2x1 = 8 banks
            tc.tile_pool(name="mmps", bufs=2, space="PSUM") as mmps,
            tc.tile_pool(name="scps", bufs=2, space="PSUM") as scps,
            tc.tile_pool(name="pvps", bufs=1, space="PSUM") as pvps,
        ):
            # ---- constants (tiles only; DMAs are interleaved with the
            # first token-block loads in the driver so the first k-proj
            # matmul is not queued behind 6us of weight transfers) ----
            w_sb_shape = [P, N_CT, 2, FW] if FP8 else [P, N_CT, FW]
            wq_sb = const.tile(w_sb_shape, IN_DT)
            wk_sb = const.tile(w_sb_shape, IN_DT)
            wv_sb = const.tile(w_sb_shape, IN_DT)
            wo_sb = const.tile([P, N_FT, N_CT, P], dt.float32r)
            mask_sb = None
            if n_patterns:
                mask_sb = const.tile([P, n_patterns, SB], dt.bfloat16)
            bias_sb = {}
            if has_bias:
                for name in ("q", "k"):
                    bias_sb[name] = const.tile(
                        [P, N_FT], dt.float32, name=f"b{name}"
                    )

            # ---- persistent activations ----
            qh_sb = big.tile([P, N_FT, NTOK_LOC], dt.float32r)
            kh_sb = big.tile([P, N_FT, NTOK_LOC], dt.float32r)
            ao_sb = big.tile([P, N_FT, NTOK_LOC], dt.float32r)
            # selector rows for the l-broadcast matmuls: sel[64, 0:64]=1
            # routes ltmp row 64 -> psum rows 0:64; sel[0, 64:128]=1 routes
            # row 0 -> rows 64:128 (K=1 matmuls, 213ns each on PE)
            sel = const.tile([P, P], dt.float32r, name="sel")
            if NORM_MODE == "mmb":
                # memset can't emit fp32r: memset an fp32 scratch, then a
                # DVE copy (a legal fp32r producer) rounds it over
                selt = const.tile([P, P], dt.float32, name="selt")
                nc.vector.memset(selt[64:65, 0:64], 1.0)
                nc.vector.memset(selt[64:65, 64:128], 0.0)
                nc.vector.memset(selt[0:1, 0:64], 0.0)
                nc.vector.memset(selt[0:1, 64:128], 1.0)
                nc.vector.tensor_copy(sel[0:1, :], selt[0:1, :])
                nc.vector.tensor_copy(sel[64:65, :], selt[64:65, :])
            # per head-pair: [t', t-tile, 192] bf16: [vh_a | ones | vh_b]
            #   h0: lhsT = cols 0:128  = [v_a | 1] -> psum [data(0:64); l(64:128)]
            #   h1: lhsT = cols 64:192 = [1 | v_b] -> psum [l(0:64); data(64:128)]
            vh_sb = [
                big.tile([P, N_TT, 192], dt.bfloat16, name=f"vh{hp}")
                for hp in range(N_HP)
            ]
            for hp in range(N_HP):
                nc.vector.memset(vh_sb[hp][:, :, 64:128], 1.0)

            def copyback(dst_ap, ps, bias_ap):
                if bias_ap is not None:
                    nc.vector.tensor_tensor(
                        dst_ap, ps, bias_ap.to_broadcast(ps.shape),
                        mybir.AluOpType.add,
                    )
                else:
                    nc.vector.tensor_copy(dst_ap, ps)

            def load_st(src, nb, tag):
                """DMA one 512-token block of one input, in four quarters
                (the ct-loop consumer starts after the first lands)."""
                nsl = slice(nb * SB, (nb + 1) * SB)
                quarters = []
                for h in range(4):
                    if FP8:
                        sth = stA.tile(
                            [P, 2, 2, SB], IN_DT, tag=f"{tag}{h}", name=f"{tag}{h}"
                        )
                        nc.sync.dma_start(
                            sth,
                            src.rearrange("(ct p) n2 -> p ct n2", p=P)[
                                :,
                                2 * h : 2 * h + 2,
                                nb * 2 * SB : (nb + 1) * 2 * SB,
                            ],
                        )
                    else:
                        sth = stA.tile(
                            [P, 2, SB], IN_DT, tag=f"{tag}{h}", name=f"{tag}{h}"
                        )
                        nc.sync.dma_start(
                            sth,
                            src[:, nsl].rearrange("(ct p) n -> p ct n", p=P)[
                                :, 2 * h : 2 * h + 2, :
                            ],
                        )
                    quarters.append(sth)
                return quarters

            DR = mybir.MatmulPerfMode.DoubleRow

            def project_qk(halves, w_sb, dst, bn, nb, ft):
                fsl = slice(ft * P, (ft + 1) * P)
                ps = mmps.tile([P, SB], dt.float32, tag="ps", name="ps")
                if FP8:
                    # G1: x8*w8 paired over each quarter's 2 k-tiles
                    for h in range(4):
                        nc.tensor.matmul(
                            ps,
                            lhsT=w_sb[:, 2 * h : 2 * h + 2, 1, fsl],
                            rhs=halves[h][:, :, 0, :],
                            start=(h == 0),
                            stop=False,
                            perf_mode=DR,
                        )
                    # G2: (wr, w8) x (x8, xr) per k-tile = x8*wr + xr*w8
                    for ct in range(N_CT):
                        nc.tensor.matmul(
                            ps,
                            lhsT=w_sb[:, ct, :, fsl],
                            rhs=halves[ct // 2][:, ct % 2, :, :],
                            start=False,
                            stop=(ct == N_CT - 1),
                            perf_mode=DR,
                        )
                else:
                    for ct in range(N_CT):
                        nc.tensor.matmul(
                            ps,
                            lhsT=w_sb[:, ct, fsl],
                            rhs=halves[ct // 2][:, ct % 2, :],
                            start=(ct == 0),
                            stop=(ct == N_CT - 1),
                        )
                bias_ap = (
                    bias_sb[bn][:, ft : ft + 1] if (has_bias and bn) else None
                )
                copyback(dst[:, ft, nb * SB : (nb + 1) * SB], ps, bias_ap)

            def project_v(halves, nb, m):
                """Project tokens [nb*SB + m*P, +P) of v directly into the
                token-major vh tiles (lhsT = the v input tile; moving dim =
                the 256 output features)."""
                tt = nb * 4 + m
                msl = slice(m * P, (m + 1) * P)
                ps = mmps.tile([P, FW], dt.float32, tag="ps", name="psv")
                if FP8:
                    for h in range(4):
                        nc.tensor.matmul(
                            ps,
                            lhsT=halves[h][:, :, 0, msl],
                            rhs=wv_sb[:, 2 * h : 2 * h + 2, 1, :],
                            start=(h == 0),
                            stop=False,
                            perf_mode=DR,
                        )
                    for ct in range(N_CT):
                        nc.tensor.matmul(
                            ps,
                            lhsT=halves[ct // 2][:, ct % 2, :, msl],
                            rhs=wv_sb[:, ct, :, :],
                            start=False,
                            stop=(ct == N_CT - 1),
                            perf_mode=DR,
                        )
                else:
                    for ct in range(N_CT):
                        nc.tensor.matmul(
                            ps,
                            lhsT=halves[ct // 2][:, ct % 2, msl],
                            rhs=wv_sb[:, ct, :],
                            start=(ct == 0),
                            stop=(ct == N_CT - 1),
                        )
                # v bias is folded on the host: softmax rows sum to 1, so
                # bv contributes the constant bv @ Wo.T to the output.
                for hp in range(N_HP):
                    nc.vector.tensor_copy(
                        vh_sb[hp][:, tt, 0:64], ps[:, hp * P : hp * P + 64]
                    )
                    nc.vector.tensor_copy(
                        vh_sb[hp][:, tt, 128:192], ps[:, hp * P + 64 : hp * P + 128]
                    )

            def flush_filler_upto(j):
                while filler and (filler[0][2] is None or filler[0][2] <= j):
                    unit, cost, _ = filler.popleft()
                    unit()

            def attention_block(hp, j):
                flush_filler_upto(j)
                ilist = [i for i in range(N_TT) if kind[i][j] != "skip"]
                assert ilist, "fully-masked s-block unsupported"
                pv = [
                    pvps.tile([P, SB], dt.float32, tag=f"pv{h}", name=f"pv{h}")
                    for h in range(2)
                ]
                ssl = slice(j * SB, (j + 1) * SB)
                n_pv = len(ilist)
                pv_emitted = 0
                deferred = []

                def emit_pv(i, e, s0):
                    nonlocal pv_emitted
                    for h in range(2):
                        nc.tensor.matmul(
                            pv[h][:, s0:],
                            lhsT=vh_sb[hp][:, i, h * 64 : h * 64 + 128],
                            rhs=e[:, h, s0:],
                            start=(pv_emitted == 0),
                            stop=(pv_emitted == n_pv - 1),
                        )
                    pv_emitted += 1

                can_trim = any(
                    kind[i][j] == "plain"
                    or (kind[i][j] == "affine" and mix_idx[i][j] == 0)
                    for i in ilist
                )

                def trim(i):
                    if can_trim and kind[i][j] == "affine":
                        return min(mix_idx[i][j], SB)
                    return 0

                # masked tiles first (their PVs are deferred so the GPSIMD
                # mask op gets the whole block); smallest trim first so the
                # first-emitted PV covers the whole block.
                ilist.sort(key=lambda i: (kind[i][j] == "plain", trim(i)))
                # plain-tile PVs are emitted one tile late so a PV waiting
                # on its exp never sits ahead of already-ready score
                # matmuls in the in-order PE stream; masked-tile PVs are
                # released once 2 further tiles have issued (the GPSIMD
                # mask is long done by then) so the block-end PV burst --
                # which delays the serial normalize chain -- stays short
                pending = None
                for it_idx, i in enumerate(ilist):
                    while deferred and deferred[0][3] <= it_idx - 2:
                        di, de, ds0, _ = deferred.pop(0)
                        emit_pv(di, de, ds0)
                    s0 = trim(i)
                    # fp32r score matmuls drop to 1/4 rate below 256 moving
                    s0_sc = min(s0, SB - 256)
                    tsl = slice(i * P, (i + 1) * P)
                    sc = scps.tile([P, 2, SB], dt.float32, tag="sc", name="sc")
                    for h in range(2):
                        hs = slice(h * 64, h * 64 + 64)
                        nc.tensor.matmul(
                            sc[:, h, s0_sc:],
                            lhsT=kh_sb[hs, hp, tsl],
                            rhs=qh_sb[hs, hp, ssl][:, s0_sc:],
                            start=True,
                            stop=True,
                        )
                            # bufs=10: affine tiles' e is held (PV deferred) until
                    # block end -- 4 held + 6 rotating in the last round
                    e = expp.tile([P, 2, SB], dt.bfloat16, tag="e", name="e", bufs=10)
                    nc.scalar.activation(
                        e[:, :, s0:],
                        sc[:, :, s0:],
                        mybir.ActivationFunctionType.Exp,
                        # fp8 weights are pre-scaled by W_SCALE on both the
                        # q and k sides; fold the (W_SCALE^2) out here
                        scale=1.0
                        / float(np.sqrt(DEPTH) * (W_SCALE**2 if FP8 else 1.0)),
                    )
                    if kind[i][j] == "affine":
                        # zero e[t', h, s'] where s' < c + t'; only the
                        # 128-col window [s0, c+128) can contain masked
                        # elements (right of it, s >= c+127 >= c+t always).
                        c = mix_idx[i][j]
                        w1 = min(SB, c + P)
                        nc.gpsimd.affine_select(
                            out=e[:, :, s0:w1],
                            in_=e[:, :, s0:w1],
                            pattern=[[0, 2], [1, w1 - s0]],
                            compare_op=mybir.AluOpType.is_ge,
                            fill=0.0,
                            base=s0 - c,
                            channel_multiplier=-1,
                        )
                        deferred.append((i, e, s0, it_idx))
                    elif kind[i][j] == "mixed":
                        u = mix_idx[i][j]
                        nc.vector.tensor_tensor(
                            e,
                            e,
                            mask_sb[:, u, None, :].to_broadcast(e.shape),
                            mybir.AluOpType.mult,
                        )
                        deferred.append((i, e, 0, it_idx))
                    else:
                        if pending is not None:
                            emit_pv(*pending)
                        pending = (i, e, 0)
                    budget[0] += 1
                    # strict pacing: at most one unit per `cost` tiles --
                    # resetting (rather than decrementing) the credit stops
                    # accrued credit from burst-draining the queue, which
                    # left the long final rounds with no PE filler
                    if filler and budget[0] >= filler[0][1]:
                        unit, cost, _ = filler.popleft()
                        budget[0] = 0
                        unit()
                if pending is not None:
                    emit_pv(*pending)
                for i, e, s0, _ in deferred:
                    emit_pv(i, e, s0)
                # normalize: all 64 l-rows of each pv are identical; move one
                # onto the data's partitions.
                lin = linp.tile([P, SB], dt.float32, tag="lin", name="lin")
                if NORM_MODE == "mmb":
                    # reciprocate one l-row of each pv straight out of psum
                    # (DVE), then a pair of K=1 selector matmuls broadcasts
                    # 1/l across the data partitions
                    ltmp = linp.tile([P, SB], dt.float32r, tag="ltmp", name="ltmp")
                    # fp32r keeps ~19 mantissa bits; 1/l at fp32r is far
                    # below the bf16 noise already in the probs
                    with nc.allow_low_precision(reason="1/l rounded to fp32r"):
                        nc.vector.reciprocal(ltmp[64:65, :], pv[0][64:65, :])
                        nc.vector.reciprocal(ltmp[0:1, :], pv[1][0:1, :])
                    bps = mmps.tile([P, SB], dt.float32, tag="ps", name="lb")
                    nc.tensor.matmul(
                        bps, lhsT=sel[64:65, :], rhs=ltmp[64:65, :],
                        start=True, stop=False,
                    )
                    nc.tensor.matmul(
                        bps, lhsT=sel[0:1, :], rhs=ltmp[0:1, :],
                        start=False, stop=True,
                    )
                    nc.vector.tensor_copy(lin, bps)
                else:
                    ltmp = linp.tile([P, SB], dt.float32, tag="ltmp", name="ltmp")
                    nc.vector.tensor_copy(ltmp[64:128, :], pv[0][64:128, :])
                    nc.vector.tensor_copy(ltmp[0:64, :], pv[1][0:64, :])
                    nc.gpsimd.dma_start(lin[0:64, :], ltmp[64:128, :])
                    nc.gpsimd.dma_start(lin[64:128, :], ltmp[0:64, :])
                    nc.vector.reciprocal(lin, lin)
                nc.vector.tensor_tensor(
                    ao_sb[0:64, hp, ssl], pv[0][0:64, :], lin[0:64, :],
                    mybir.AluOpType.mult,
                )
                nc.vector.tensor_tensor(
                    ao_sb[64:128, hp, ssl], pv[1][64:128, :], lin[64:128, :],
                    mybir.AluOpType.mult,
                )

            def outproj_nb(nb, dgroup, kts=(0, 1), out_col0=None):
                """Output projection for n-block nb, d-tiles [2*dgroup, +2),
                contracting feature-tiles `kts`, writing token-cols at
                out_col0 (defaults to the block's own columns)."""
                nsl = slice(nb * SB, (nb + 1) * SB)
                if out_col0 is None:
                    out_col0 = nb * SB
                osl = slice(out_col0, out_col0 + SB)
                ost = expp.tile([P, 2, SB], dt.bfloat16, tag="ost", name="ost", bufs=3)
                for u in range(2):
                    dtile = 2 * dgroup + u
                    ps = mmps.tile([P, SB], dt.float32, tag="ps", name="po")
                    for ki, kt in enumerate(kts):
                        nc.tensor.matmul(
                            ps,
                            lhsT=wo_sb[:, kt, dtile, :],
                            rhs=ao_sb[:, kt, nsl],
                            start=(ki == 0),
                            stop=(ki == len(kts) - 1),
                        )
                    # GPSIMD can't read psum; DVE has the headroom
                    nc.vector.tensor_copy(ost[:, u, :], ps)
                nc.sync.dma_start(
                    outT_r[:, 2 * dgroup : 2 * dgroup + 2, osl], ost
                )

            # ---- driver ----
            from collections import deque

            filler = deque()
            budget = [0]
            reserve = (2, 3)  # nb2 outproj dgroups held for the drain window

            def push_block_units(nb):
                kh_halves = [None]
                qh_halves = [None]
                vh_halves = [None]

                def dma_unit(src, tag, store):
                    def u():
                        store[0] = load_st(src, nb, tag)
                    return u

                filler.append((dma_unit(kT, "sk", kh_halves), C_DMA, nb))
                filler.append((dma_unit(qT, "sq", qh_halves), C_DMA, nb))
                filler.append((dma_unit(vT, "sv", vh_halves), C_DMA, nb))
                for ft in range(N_FT):
                    filler.append(
                        (lambda f=ft: project_qk(kh_halves[0], wk_sb, kh_sb, "k", nb, f), C_QK, nb)
                    )
                for ft in range(N_FT):
                    filler.append(
                        (lambda f=ft: project_qk(qh_halves[0], wq_sb, qh_sb, "q", nb, f), C_QK, nb)
                    )
                for m in range(4):
                    filler.append(
                        (lambda x=m: project_v(vh_halves[0], nb, x), C_V, nb)
                    )

            # block (·, 0) prerequisites run serially; everything later is
            # interleaved into the attention stream.  DMA issue order
            # interleaves weight halves with token halves so the first
            # k-proj matmul (needs wk half 0 + k-tokens half 0) starts
            # ~3us in.
            def w_rearr(w):
                if FP8:
                    return w.rearrange("(ct p) (two f) -> p ct two f", p=P, two=2)
                return w.rearrange("(ct p) f -> p ct f", p=P)

            def load_w(w_sb, w):
                wr = w_rearr(w)
                for h in range(2):
                    nc.sync.dma_start(
                        w_sb[:, 4 * h : 4 * h + 4], wr[:, 4 * h : 4 * h + 4]
                    )

            wkr = w_rearr(wk)
            kh0 = []
            for h in range(4):
                nc.sync.dma_start(
                    wk_sb[:, 2 * h : 2 * h + 2], wkr[:, 2 * h : 2 * h + 2]
                )
                if FP8:
                    sth = stA.tile([P, 2, 2, SB], IN_DT, tag=f"sk{h}", name=f"sk{h}")
                    nc.sync.dma_start(
                        sth,
                        kT.rearrange("(ct p) n2 -> p ct n2", p=P)[
                            :, 2 * h : 2 * h + 2, 0 : 2 * SB
                        ],
                    )
                else:
                    sth = stA.tile([P, 2, SB], IN_DT, tag=f"sk{h}", name=f"sk{h}")
                    nc.sync.dma_start(
                        sth,
                        kT[:, 0:SB].rearrange("(ct p) n -> p ct n", p=P)[
                            :, 2 * h : 2 * h + 2
                        ],
                    )
                kh0.append(sth)
            load_w(wq_sb, wq)
            qh0 = load_st(qT, 0, "sq")
            load_w(wv_sb, wv)
            vh0 = load_st(vT, 0, "sv")
            nc.sync.dma_start(
                wo_sb, wo.rearrange("(kt p) (dt q) -> p kt dt q", p=P, q=P)
            )
            if n_patterns:
                nc.sync.dma_start(mask_sb, maskt.rearrange("m p s -> p m s"))
            if has_bias:
                nc.sync.dma_start(
                    bias_sb["q"], bq.rearrange("(ft p) o -> p (ft o)", p=P)
                )
                nc.sync.dma_start(
                    bias_sb["k"], bk.rearrange("(ft p) o -> p (ft o)", p=P)
                )
            for ft in range(N_FT):
                project_qk(kh0, wk_sb, kh_sb, "k", 0, ft)
            for ft in range(N_FT):
                project_qk(qh0, wq_sb, qh_sb, "q", 0, ft)
            for m in range(4):
                project_v(vh0, 0, m)
            for nb in range(1, N_NB):
                push_block_units(nb)
            last = N_SB - 1
            for j in range(N_SB):
                attention_block(0, j)
                if j == last:
                    # kt=0 half of the last block's outproj runs as filler
                    # during the final attention block; kt=1 follows after
                    # (host sums the two partials).  Two nb2 units are held
                    # in reserve behind them (cost 99 blocks budget-drain)
                    # to keep PE hot through the final normalize chain.
                    for dg in range(N_CT // 2):
                        filler.append(
                            (lambda g=dg: outproj_nb(last, g, kts=(0,)), 1, None)
                        )
                    for g in reserve:
                        # key 999 is never force-flushed; the huge cost is
                        # never budget-drained -- these pop only in the
                        # final drain loop, after the last normalize is
                        # emitted, keeping PE hot through that chain
                        filler.append(
                            (lambda x=g: outproj_nb(last - 1, x), 10**9, 999)
                        )
                attention_block(1, j)
                if j == last:
                    for dg in range(N_CT // 2):
                        filler.append(
                            (
                                lambda g=dg: outproj_nb(
                                    last, g, kts=(1,), out_col0=NTOK_LOC
                                ),
                                1,
                                None,
                            )
                        )
                else:
                    dgs = range(N_CT // 2)
                    if j == last - 1:
                        dgs = range(N_CT // 2 - len(reserve))
                    for dg in dgs:
                        # higher cost spreads outproj into the
                        # (filler-starved) late attention rounds
                        filler.append((lambda x=j, g=dg: outproj_nb(x, g), C_OP, None))
            while filler:
                filler.popleft()[0]()

    nc.compile()
    return nc


_NC_CACHE = {}


def _get_nc(kind_key, kind, mix_idx, n_patterns, has_bias):
    key = (kind_key, n_patterns, has_bias)
    if key not in _NC_CACHE:
        _NC_CACHE[key] = _build_nc(kind, mix_idx, n_patterns, has_bias)
    return _NC_CACHE[key]


F8 = ml_dtypes.float8_e4m3


def _split8(x):
    """fp32 [D, N] -> [D, 2N] fp8e4 with (hi, lo) interleaved per
    512-token block: layout [D, nb, {hi, lo}, SB]."""
    d = x.shape[0]
    hi = x.astype(F8)
    lo = (x - hi.astype(np.float32)).astype(F8)
    a = np.stack(
        [hi.reshape(d, -1, SB), lo.reshape(d, -1, SB)], axis=2
    )  # [D, nb, 2, SB]
    return np.ascontiguousarray(a.reshape(d, -1))


def _splitw8(wT):
    """Weight [D, F] fp32, pre-scaled: -> [D, 2F] fp8 in (lo, hi) order."""
    ws = wT * W_SCALE
    hi = ws.astype(F8)
    lo = (ws - hi.astype(np.float32)).astype(F8)
    return np.ascontiguousarray(
        np.stack([lo, hi], axis=1).reshape(wT.shape[0], -1)
    )


def kernel(v, k, q, mask, Wq, bq, Wk, bk, Wv, bv, Wo, bo, trace=False):
    global LAST_RESULTS, LAST_EXEC_WALL
    from concourse.bass_utils import run_bass_kernel_spmd

    in_np = ml_dtypes.bfloat16

    def prep_T(x):  # [S, D] -> [D, S] in input dtype (or fp8 hi/lo pair)
        xT = np.ascontiguousarray(np.asarray(x, dtype=np.float32).T)
        if FP8:
            return _split8(xT)
        return xT.astype(in_np)

    kind, mix_idx, patterns = _mask_structure(np.asarray(mask, dtype=np.float32))
    maskt = (
        np.ascontiguousarray(np.stack(patterns)).astype(ml_dtypes.bfloat16)
        if patterns
        else None
    )

    has_bias = bool(np.any(np.asarray(bq)) or np.any(np.asarray(bk)))
    kind_key = str(kind) + str(mix_idx)
    nc = _get_nc(kind_key, kind, mix_idx, len(patterns), has_bias)

    q_np = np.asarray(q, np.float32)
    k_np = np.asarray(k, np.float32)
    v_np = np.asarray(v, np.float32)
    qT = [prep_T(q_np[b]) for b in range(BATCH)]
    kT = [prep_T(k_np[b]) for b in range(BATCH)]
    vT = [prep_T(v_np[b]) for b in range(BATCH)]

    in_maps = []
    for core in range(N_CORES):
        b = core // 4
        hg = core % 4
        fsl = slice(hg * FW, (hg + 1) * FW)
        def prep_w(W):
            wT = np.ascontiguousarray(np.asarray(W, np.float32)[fsl].T)
            if FP8:
                return _splitw8(wT)
            return wT.astype(in_np)

        wo_scale = W_SCALE if FP8 else 1.0
        m = {
            "qT": qT[b],
            "kT": kT[b],
            "vT": vT[b],
            "wqT": prep_w(Wq),
            "wkT": prep_w(Wk),
            "wvT": prep_w(Wv),
            "woT": np.ascontiguousarray(
                np.asarray(Wo, np.float32)[:, fsl].T / wo_scale
            ),
        }
        if maskt is not None:
            m["maskt"] = maskt
        if has_bias:
            # projections are scaled by W_SCALE in fp8 mode; scale the
            # biases to match (the exp scale folds it back out)
            m["bq"] = np.asarray(bq, np.float32)[fsl].reshape(FW, 1) * wo_scale
            m["bk"] = np.asarray(bk, np.float32)[fsl].reshape(FW, 1) * wo_scale
        in_maps.append(m)

    import time as _time

    _t0 = _time.time()
    res = run_bass_kernel_spmd(
        nc, in_maps, core_ids=list(range(N_CORES)), trace=trace
    )
    LAST_EXEC_WALL = _time.time() - _t0
    LAST_RESULTS = res

    out = np.zeros((BATCH, SEQ, D_MODEL), dtype=np.float32)
    last0 = (N_NB - 1) * SB
    for core in range(N_CORES):
        b = core // 4
        oT = res.results[core]["outT"]
        out[b] += oT[:, :NTOK_LOC].T
        # kt=1 partial of the last n-block lives in the extra columns
        out[b, last0 : last0 + SB] += oT[:, NTOK_LOC:].T
    # v-bias contributes the constant bv @ Wo.T (softmax rows sum to 1)
    out += (
        np.asarray(bo, np.float32)
        + np.asarray(bv, np.float32) @ np.asarray(Wo, np.float32).T
    )[None, None, :]
    return out


# revision 89
# speedup vs baseline: 1.3542x; 1.0148x over previous
"""Multi-head attention (B=2, S=2048, D=1024, H=16) on 8 Trainium2 cores.

Sharding: 2-way data parallel over batch x 4-way tensor parallel over heads.
Each core owns one batch and 4 heads (a 256-feature slice).  Per core:
  - QKV projections for its feature slice over its batch's 2048 tokens
  - causal attention for its 4 heads with block-skipping
  - partial output projection (contraction over its 256 features)
Host: transposes/prepares inputs, sums the 4 partial outputs per batch, adds
bo.

On-chip pipeline (vs the previous head-only-TP kernel):
  - v is projected DIRECTLY into token-major layout (lhsT = the v input
    tile, moving dim = features), eliminating the PE transposes and their
    vector-engine copybacks.
  - the output projection DMAs fp32 psum straight to DRAM (no psum->SBUF
    copy, no bf16 cast); the host sums fp32 partials.
  - the causal mask affine_select only touches the 128-column window that
    straddles the diagonal (everything right of it is fully kept).
  - softmax row-sums come for free from ones-columns in the PV lhsT; the
    cross-partition bounce uses GPSIMD partition_broadcast (or an SBUF DMA
    fallback, MHA_NORM=dma).
  - q/k projections are stored float32r; score matmuls run float32r at
    full PE rate (moving dim >= 256).  v/probs are bf16.
Cross-phase work (next-block projections, finished-block output
projections) is interleaved into the attention stream with budget pacing.
"""

import os

import numpy as np
import ml_dtypes

D_MODEL = 1024
NUM_HEADS = 16
DEPTH = 64
BATCH = 2
SEQ = 2048
N_CORES = 8
H_LOC = 4  # heads per core
FW = 256  # features per core (4 heads x 64)
P = 128
SB = 512  # s-block width
NTOK_LOC = SEQ  # tokens per core (one batch)
N_SB = SEQ // SB  # 4 s-blocks
N_TT = SEQ // P  # 16 t-tiles
N_NB = NTOK_LOC // SB  # 4 n-blocks
N_CT = D_MODEL // P  # 8 contraction tiles
N_FT = FW // P  # 2 feature tiles
N_HP = H_LOC // 2  # 2 head-pairs

NORM_MODE = os.environ.get("MHA_NORM", "mmb")  # mmb | dma
# fp8 3-term projections: inputs/weights are split hi+lo into fp8e4 on the
# host; each 128-deep contraction tile needs 3 of the 4 cross products
# (x8*w8, x8*wr, xr*w8 -- the dropped xr*wr term is O(quant_err^2)), and
# DoubleRow packs 2 products per matmul at 0.5 cycles/row: 12 instructions
# replace 8 at 0.75x the PE time with ~2x BETTER accuracy than bf16 inputs.
# Weights are pre-scaled by W_SCALE so their residuals stay out of the
# fp8 subnormal range; the scale is folded into the exp scale and Wo.
FP8 = os.environ.get("MHA_FP8", "1") == "1"
W_SCALE = 32.0
# filler pacing costs (in score-tile-iteration credits)
C_DMA = int(os.environ.get("MHA_C_DMA", "1"))
C_QK = int(os.environ.get("MHA_C_QK", "3"))
C_V = int(os.environ.get("MHA_C_V", "2"))
C_OP = int(os.environ.get("MHA_C_OP", "3"))

LAST_RESULTS = None
LAST_EXEC_WALL = None


def _mask_structure(mask_np):
    """Classify each (t-tile, s-block) of the [S, S] mask (1.0 = disallowed).

    Returns (kind, mix_idx, patterns): kind[i][j] in
    {"skip", "plain", "affine", "mixed"}; for "affine", mix_idx[i][j] is the
    offset c of keep = (s >= c + t); for "mixed" it indexes into patterns
    (list of [P, SB] keep-masks).  mask rows = query s, cols = key t;
    scoresT is [t, s] so we transpose.
    """
    maskT = np.ascontiguousarray(mask_np.reshape(SEQ, SEQ).T)
    kind = [[None] * N_SB for _ in range(N_TT)]
    mix_idx = [[None] * N_SB for _ in range(N_TT)]
    patterns = []
    pat_key = {}
    s_idx = np.arange(SB)[None, :]
    t_idx = np.arange(P)[:, None]
    for i in range(N_TT):
        for j in range(N_SB):
            sub = maskT[i * P : (i + 1) * P, j * SB : (j + 1) * SB]
            if np.all(sub >= 0.5):
                kind[i][j] = "skip"
                continue
            if np.all(sub < 0.5):
                kind[i][j] = "plain"
                continue
            keep = (sub < 0.5).astype(np.float32)
            first_one = np.argmax(keep, axis=1)
            c = int(first_one[0])
            if np.array_equal(keep, (s_idx >= c + t_idx).astype(np.float32)):
                kind[i][j] = "affine"
                mix_idx[i][j] = c
                continue
            kind[i][j] = "mixed"
            key = keep.tobytes()
            if key not in pat_key:
                pat_key[key] = len(patterns)
                patterns.append(keep)
            mix_idx[i][j] = pat_key[key]
    return kind, mix_idx, patterns


def _build_nc(kind, mix_idx, n_patterns, has_bias):
    import concourse.tile as tile
    import concourse.mybir as mybir
    from concourse import bacc

    dt = mybir.dt

    nc = bacc.Bacc(None, target_bir_lowering=False)

    if FP8:
        # inputs ship as (hi, lo) fp8e4 pairs interleaved per 512-token
        # block (so DMA slices stay <= 3 dims); weights interleave
        # (lo, hi) at full width.  The orders make the DoubleRow slot APs
        # of all three product groups regular-stride slices.
        IN_DT = dt.float8e4
        in_shape = [D_MODEL, 2 * NTOK_LOC]
        w_shape = [D_MODEL, 2 * FW]
    else:
        IN_DT = dt.bfloat16
        in_shape = [D_MODEL, NTOK_LOC]
        w_shape = [D_MODEL, FW]
    qT = nc.dram_tensor("qT", in_shape, IN_DT, kind="ExternalInput")
    kT = nc.dram_tensor("kT", in_shape, IN_DT, kind="ExternalInput")
    vT = nc.dram_tensor("vT", in_shape, IN_DT, kind="ExternalInput")
    wq = nc.dram_tensor("wqT", w_shape, IN_DT, kind="ExternalInput")
    wk = nc.dram_tensor("wkT", w_shape, IN_DT, kind="ExternalInput")
    wv = nc.dram_tensor("wvT", w_shape, IN_DT, kind="ExternalInput")
    wo = nc.dram_tensor("woT", [FW, D_MODEL], dt.float32r, kind="ExternalInput")
    maskt = None
    if n_patterns:
        maskt = nc.dram_tensor(
            "maskt", [n_patterns, P, SB], dt.bfloat16, kind="ExternalInput"
        )
    bq = bk = bv = None
    if has_bias:
        bq = nc.dram_tensor("bq", [FW, 1], dt.float32, kind="ExternalInput")
        bk = nc.dram_tensor("bk", [FW, 1], dt.float32, kind="ExternalInput")
        bv = nc.dram_tensor("bv", [FW, 1], dt.float32, kind="ExternalInput")
    # cols [NTOK_LOC, NTOK_LOC+SB) hold the kt=1 partial of the last
    # n-block (its outproj is split by feature-half so the first half can
    # run during the final attention block); host sums the two partials.
    outT = nc.dram_tensor(
        "outT", [D_MODEL, NTOK_LOC + SB], dt.bfloat16, kind="ExternalOutput"
    )
    outT_r = outT.rearrange("(dt p) n -> p dt n", p=P)

    with tile.TileContext(nc) as tc:
        with (
            tc.tile_pool(name="const", bufs=1) as const,
            tc.tile_pool(name="big", bufs=1) as big,
            tc.tile_pool(name="stageA", bufs=2) as stA,
            tc.tile_pool(name="expp", bufs=6) as expp,
            tc.tile_pool(name="linp", bufs=2) as linp,
            # PSUM: mm 2x1 banks + sc 2x2 banks + pv 2x1 = 8 banks
            tc.tile_pool(name="mmps", bufs=2, space="PSUM") as mmps,
            tc.tile_pool(name="scps", bufs=2, space="PSUM") as scps,
            tc.tile_pool(name="pvps", bufs=1, space="PSUM") as pvps,
        ):
            # ---- constants (tiles only; DMAs are interleaved with the
            # first token-block loads in the driver so the first k-proj
            # matmul is not queued behind 6us of weight transfers) ----
            w_sb_shape = [P, N_CT, 2, FW] if FP8 else [P, N_CT, FW]
            wq_sb = const.tile(w_sb_shape, IN_DT)
            wk_sb = const.tile(w_sb_shape, IN_DT)
            wv_sb = const.tile(w_sb_shape, IN_DT)
            wo_sb = const.tile([P, N_FT, N_CT, P], dt.float32r)
            mask_sb = None
            if n_patterns:
                mask_sb = const.tile([P, n_patterns, SB], dt.bfloat16)
            bias_sb = {}
            if has_bias:
                for name in ("q", "k"):
                    bias_sb[name] = const.tile(
                        [P, N_FT], dt.float32, name=f"b{name}"
                    )

            # ---- persistent activations ----
            qh_sb = big.tile([P, N_FT, NTOK_LOC], dt.float32r)
            kh_sb = big.tile([P, N_FT, NTOK_LOC], dt.float32r)
            ao_sb = big.tile([P, N_FT, NTOK_LOC], dt.float32r)
            # selector rows for the l-broadcast matmuls: sel[64, 0:64]=1
            # routes ltmp row 64 -> psum rows 0:64; sel[0, 64:128]=1 routes
            # row 0 -> rows 64:128 (K=1 matmuls, 213ns each on PE)
            sel = const.tile([P, P], dt.float32r, name="sel")
            if NORM_MODE == "mmb":
                # memset can't emit fp32r: memset an fp32 scratch, then a
                # DVE copy (a legal fp32r producer) rounds it over
                selt = const.tile([P, P], dt.float32, name="selt")
                nc.vector.memset(selt[64:65, 0:64], 1.0)
                nc.vector.memset(selt[64:65, 64:128], 0.0)
                nc.vector.memset(selt[0:1, 0:64], 0.0)
                nc.vector.memset(selt[0:1, 64:128], 1.0)
                nc.vector.tensor_copy(sel[0:1, :], selt[0:1, :])
                nc.vector.tensor_copy(sel[64:65, :], selt[64:65, :])
            # per head-pair: [t', t-tile, 192] bf16: [vh_a | ones | vh_b]
            #   h0: lhsT = cols 0:128  = [v_a | 1] -> psum [data(0:64); l(64:128)]
            #   h1: lhsT = cols 64:192 = [1 | v_b] -> psum [l(0:64); data(64:128)]
            vh_sb = [
                big.tile([P, N_TT, 192], dt.bfloat16, name=f"vh{hp}")
                for hp in range(N_HP)
            ]
            for hp in range(N_HP):
                nc.vector.memset(vh_sb[hp][:, :, 64:128], 1.0)

            def copyback(dst_ap, ps, bias_ap):
                if bias_ap is not None:
                    nc.vector.tensor_tensor(
                        dst_ap, ps, bias_ap.to_broadcast(ps.shape),
                        mybir.AluOpType.add,
                    )
                else:
                    nc.vector.tensor_copy(dst_ap, ps)

            def load_st(src, nb, tag):
                """DMA one 512-token block of one input, in four quarters
                (the ct-loop consumer starts after the first lands)."""
                nsl = slice(nb * SB, (nb + 1) * SB)
                quarters = []
                for h in range(4):
                    if FP8:
                        sth = stA.tile(
                            [P, 2, 2, SB], IN_DT, tag=f"{tag}{h}", name=f"{tag}{h}"
                        )
                        nc.sync.dma_start(
                            sth,
                            src.rearrange("(ct p) n2 -> p ct n2", p=P)[
                                :,
                                2 * h : 2 * h + 2,
                                nb * 2 * SB : (nb + 1) * 2 * SB,
                            ],
                        )
                    else:
                        sth = stA.tile(
                            [P, 2, SB], IN_DT, tag=f"{tag}{h}", name=f"{tag}{h}"
                        )
                        nc.sync.dma_start(
                            sth,
                            src[:, nsl].rearrange("(ct p) n -> p ct n", p=P)[
                                :, 2 * h : 2 * h + 2, :
                            ],
                        )
                    quarters.append(sth)
                return quarters

            DR = mybir.MatmulPerfMode.DoubleRow

            def project_qk(halves, w_sb, dst, bn, nb, ft):
                fsl = slice(ft * P, (ft + 1) * P)
                ps = mmps.tile([P, SB], dt.float32, tag="ps", name="ps")
                if FP8:
                    # G1: x8*w8 paired over each quarter's 2 k-tiles
                    for h in range(4):
                        nc.tensor.matmul(
                            ps,
                            lhsT=w_sb[:, 2 * h : 2 * h + 2, 1, fsl],
                            rhs=halves[h][:, :, 0, :],
                            start=(h == 0),
                            stop=False,
                            perf_mode=DR,
                        )
                    # G2: (wr, w8) x (x8, xr) per k-tile = x8*wr + xr*w8
                    for ct in range(N_CT):
                        nc.tensor.matmul(
                            ps,
                            lhsT=w_sb[:, ct, :, fsl],
                            rhs=halves[ct // 2][:, ct % 2, :, :],
                            start=False,
                            stop=(ct == N_CT - 1),
                            perf_mode=DR,
                        )
                else:
                    for ct in range(N_CT):
                        nc.tensor.matmul(
                            ps,
                            lhsT=w_sb[:, ct, fsl],
                            rhs=halves[ct // 2][:, ct % 2, :],
                            start=(ct == 0),
                            stop=(ct == N_CT - 1),
                        )
                bias_ap = (
                    bias_sb[bn][:, ft : ft + 1] if (has_bias and bn) else None
                )
                copyback(dst[:, ft, nb * SB : (nb + 1) * SB], ps, bias_ap)

            def project_v(halves, nb, m):
                """Project tokens [nb*SB + m*P, +P) of v directly into the
                token-major vh tiles (lhsT = the v input tile; moving dim =
                the 256 output features)."""
                tt = nb * 4 + m
                msl = slice(m * P, (m + 1) * P)
                ps = mmps.tile([P, FW], dt.float32, tag="ps", name="psv")
                if FP8:
                    for h in range(4):
                        nc.tensor.matmul(
                            ps,
                            lhsT=halves[h][:, :, 0, msl],
                            rhs=wv_sb[:, 2 * h : 2 * h + 2, 1, :],
                            start=(h == 0),
                            stop=False,
                            perf_mode=DR,
                        )
                    for ct in range(N_CT):
                        nc.tensor.matmul(
                            ps,
                            lhsT=halves[ct // 2][:, ct % 2, :, msl],
                            rhs=wv_sb[:, ct, :, :],
                            start=False,
                            stop=(ct == N_CT - 1),
                            perf_mode=DR,
                        )
                else:
                    for ct in range(N_CT):
                        nc.tensor.matmul(
                            ps,
                            lhsT=halves[ct // 2][:, ct % 2, msl],
                            rhs=wv_sb[:, ct, :],
                            start=(ct == 0),
                            stop=(ct == N_CT - 1),
                        )
                # v bias is folded on the host: softmax rows sum to 1, so
                # bv contributes the constant bv @ Wo.T to the output.
                for hp in range(N_HP):
                    nc.vector.tensor_copy(
                        vh_sb[hp][:, tt, 0:64], ps[:, hp * P : hp * P + 64]
                    )
                    nc.vector.tensor_copy(
                        vh_sb[hp][:, tt, 128:192], ps[:, hp * P + 64 : hp * P + 128]
                    )

            def flush_filler_upto(key):
                while filler and (filler[0][2] is None or filler[0][2] <= key):
                    unit, cost, _ = filler.popleft()
                    unit()

            def attention_block(hp, j):
                flush_filler_upto((j, hp))
                ilist = [i for i in range(N_TT) if kind[i][j] != "skip"]
                assert ilist, "fully-masked s-block unsupported"
                pv = [
                    pvps.tile([P, SB], dt.float32, tag=f"pv{h}", name=f"pv{h}")
                    for h in range(2)
                ]
                ssl = slice(j * SB, (j + 1) * SB)
                n_pv = len(ilist)
                pv_emitted = 0
                deferred = []

                def emit_pv(i, e, s0):
                    nonlocal pv_emitted
                    for h in range(2):
                        nc.tensor.matmul(
                            pv[h][:, s0:],
                            lhsT=vh_sb[hp][:, i, h * 64 : h * 64 + 128],
                            rhs=e[:, h, s0:],
                            start=(pv_emitted == 0),
                            stop=(pv_emitted == n_pv - 1),
                        )
                    pv_emitted += 1

                can_trim = any(
                    kind[i][j] == "plain"
                    or (kind[i][j] == "affine" and mix_idx[i][j] == 0)
                    for i in ilist
                )

                def trim(i):
                    if can_trim and kind[i][j] == "affine":
                        return min(mix_idx[i][j], SB)
                    return 0

                # masked tiles first (their PVs are deferred so the GPSIMD
                # mask op gets the whole block); smallest trim first so the
                # first-emitted PV covers the whole block.
                ilist.sort(key=lambda i: (kind[i][j] == "plain", trim(i)))
                # plain-tile PVs are emitted one tile late so a PV waiting
                # on its exp never sits ahead of already-ready score
                # matmuls in the in-order PE stream; masked-tile PVs are
                # released once 2 further tiles have issued (the GPSIMD
                # mask is long done by then) so the block-end PV burst --
                # which delays the serial normalize chain -- stays short
                pending = None
                for it_idx, i in enumerate(ilist):
                    while deferred and deferred[0][3] <= it_idx - 2:
                        di, de, ds0, _ = deferred.pop(0)
                        emit_pv(di, de, ds0)
                    s0 = trim(i)
                    # fp32r score matmuls drop to 1/4 rate below 256 moving
                    s0_sc = min(s0, SB - 256)
                    tsl = slice(i * P, (i + 1) * P)
                    sc = scps.tile([P, 2, SB], dt.float32, tag="sc", name="sc")
                    for h in range(2):
                        hs = slice(h * 64, h * 64 + 64)
                        nc.tensor.matmul(
                            sc[:, h, s0_sc:],
                            lhsT=kh_sb[hs, hp, tsl],
                            rhs=qh_sb[hs, hp, ssl][:, s0_sc:],
                            start=True,
                            stop=True,
                        )
                            # bufs=10: affine tiles' e is held (PV deferred) until
                    # block end -- 4 held + 6 rotating in the last round
                    e = expp.tile([P, 2, SB], dt.bfloat16, tag="e", name="e", bufs=10)
                    nc.scalar.activation(
                        e[:, :, s0:],
                        sc[:, :, s0:],
                        mybir.ActivationFunctionType.Exp,
                        # fp8 weights are pre-scaled by W_SCALE on both the
                        # q and k sides; fold the (W_SCALE^2) out here
                        scale=1.0
                        / float(np.sqrt(DEPTH) * (W_SCALE**2 if FP8 else 1.0)),
                    )
                    if kind[i][j] == "affine":
                        # zero e[t', h, s'] where s' < c + t'; only the
                        # 128-col window [s0, c+128) can contain masked
                        # elements (right of it, s >= c+127 >= c+t always).
                        c = mix_idx[i][j]
                        w1 = min(SB, c + P)
                        nc.gpsimd.affine_select(
                            out=e[:, :, s0:w1],
                            in_=e[:, :, s0:w1],
                            pattern=[[0, 2], [1, w1 - s0]],
                            compare_op=mybir.AluOpType.is_ge,
                            fill=0.0,
                            base=s0 - c,
                            channel_multiplier=-1,
                        )
                        deferred.append((i, e, s0, it_idx))
                    elif kind[i][j] == "mixed":
                        u = mix_idx[i][j]
                        nc.vector.tensor_tensor(
                            e,
                            e,
                            mask_sb[:, u, None, :].to_broadcast(e.shape),
                            mybir.AluOpType.mult,
                        )
                        deferred.append((i, e, 0, it_idx))
                    else:
                        if pending is not None:
                            emit_pv(*pending)
                        pending = (i, e, 0)
                    budget[0] += 1
                    # strict pacing: at most one unit per `cost` tiles --
                    # resetting (rather than decrementing) the credit stops
                    # accrued credit from burst-draining the queue, which
                    # left the long final rounds with no PE filler
                    if filler and budget[0] >= filler[0][1]:
                        unit, cost, _ = filler.popleft()
                        budget[0] = 0
                        unit()
                if pending is not None:
                    emit_pv(*pending)
                for i, e, s0, _ in deferred:
                    emit_pv(i, e, s0)
                is_final = (hp, j) == (1, N_SB - 1)
                if is_final:
                    # emit the reserved outproj units BEFORE the normalize:
                    # per-engine order is fixed at schedule time, so only
                    # work emitted here can keep PE busy through the final
                    # normalize chain
                    flush_filler_upto((1000, 0))
                # normalize: all 64 l-rows of each pv are identical; move one
                # onto the data's partitions.
                if NORM_MODE == "mmb":
                    # reciprocate one l-row of each pv straight out of psum
                    # (DVE), then a pair of K=1 selector matmuls broadcasts
                    # 1/l across the data partitions
                    ltmp = linp.tile([P, SB], dt.float32r, tag="ltmp", name="ltmp")
                    # fp32r keeps ~19 mantissa bits; 1/l at fp32r is far
                    # below the bf16 noise already in the probs
                    with nc.allow_low_precision(reason="1/l rounded to fp32r"):
                        nc.vector.reciprocal(ltmp[64:65, :], pv[0][64:65, :])
                        nc.vector.reciprocal(ltmp[0:1, :], pv[1][0:1, :])
                    bps = mmps.tile([P, SB], dt.float32, tag="ps", name="lb")
                    nc.tensor.matmul(
                        bps, lhsT=sel[64:65, :], rhs=ltmp[64:65, :],
                        start=True, stop=False,
                    )
                    nc.tensor.matmul(
                        bps, lhsT=sel[0:1, :], rhs=ltmp[0:1, :],
                        start=False, stop=True,
                    )
                    if is_final:
                        # nothing competes for the mmps ring after this --
                        # skip the SBUF bounce to shorten the drain chain
                        lin = bps
                    else:
                        # bounce 1/l to SBUF so the mmps slot frees
                        # immediately (TTs reading bps directly held the
                        # 2-deep ring ~2us, stalling interleaved psums)
                        lin = linp.tile([P, SB], dt.float32, tag="lin", name="lin")
                        nc.vector.tensor_copy(lin, bps)
                else:
                    lin = linp.tile([P, SB], dt.float32, tag="lin", name="lin")
                    ltmp = linp.tile([P, SB], dt.float32, tag="ltmp", name="ltmp")
                    nc.vector.tensor_copy(ltmp[64:128, :], pv[0][64:128, :])
                    nc.vector.tensor_copy(ltmp[0:64, :], pv[1][0:64, :])
                    nc.gpsimd.dma_start(lin[0:64, :], ltmp[64:128, :])
                    nc.gpsimd.dma_start(lin[64:128, :], ltmp[0:64, :])
                    nc.vector.reciprocal(lin, lin)
                nc.vector.tensor_tensor(
                    ao_sb[0:64, hp, ssl], pv[0][0:64, :], lin[0:64, :],
                    mybir.AluOpType.mult,
                )
                nc.vector.tensor_tensor(
                    ao_sb[64:128, hp, ssl], pv[1][64:128, :], lin[64:128, :],
                    mybir.AluOpType.mult,
                )

            def outproj_nb(nb, dgroup, kts=(0, 1), out_col0=None, tail=False):
                """Output projection for n-block nb, d-tiles [2*dgroup, +2),
                contracting feature-tiles `kts`, writing token-cols at
                out_col0 (defaults to the block's own columns).  In the
                drain phase (tail=True) psums come from the freed score
                pool (3 deep) and half the copies go to the Activation
                engine -- neither has attention work left there, and DVE
                alone was pacing the drain."""
                nsl = slice(nb * SB, (nb + 1) * SB)
                if out_col0 is None:
                    out_col0 = nb * SB
                osl = slice(out_col0, out_col0 + SB)
                ost = expp.tile([P, 2, SB], dt.bfloat16, tag="ost", name="ost", bufs=3)
                for u in range(2):
                    dtile = 2 * dgroup + u
                    if tail and u == 0:
                        # borrow the freed score ring so the drain runs a
                        # 4-deep psum pipeline instead of 2
                        ps = scps.tile([P, SB], dt.float32, tag="sc", name="po")
                    else:
                        ps = mmps.tile([P, SB], dt.float32, tag="ps", name="po")
                    for ki, kt in enumerate(kts):
                        nc.tensor.matmul(
                            ps,
                            lhsT=wo_sb[:, kt, dtile, :],
                            rhs=ao_sb[:, kt, nsl],
                            start=(ki == 0),
                            stop=(ki == len(kts) - 1),
                        )
                    # GPSIMD can't read psum; split Act/DVE in the tail
                    if tail and u == 0:
                        nc.scalar.copy(ost[:, u, :], ps)
                    else:
                        nc.vector.tensor_copy(ost[:, u, :], ps)
                    if tail:
                        # per-dtile DMA starts as soon as its copy lands
                        nc.sync.dma_start(outT_r[:, dtile, osl], ost[:, u, :])
                if not tail:
                    nc.sync.dma_start(
                        outT_r[:, 2 * dgroup : 2 * dgroup + 2, osl], ost
                    )

            # ---- driver ----
            from collections import deque

            filler = deque()
            budget = [0]
            reserve = (2, 3)  # nb2 outproj dgroups held for the drain window

            def push_block_units(nb):
                kh_halves = [None]
                qh_halves = [None]
                vh_halves = [None]

                def dma_unit(src, tag, store):
                    def u():
                        store[0] = load_st(src, nb, tag)
                    return u

                key = (nb, 0)
                filler.append((dma_unit(kT, "sk", kh_halves), C_DMA, key))
                filler.append((dma_unit(qT, "sq", qh_halves), C_DMA, key))
                filler.append((dma_unit(vT, "sv", vh_halves), C_DMA, key))
                for ft in range(N_FT):
                    filler.append(
                        (lambda f=ft: project_qk(kh_halves[0], wk_sb, kh_sb, "k", nb, f), C_QK, key)
                    )
                for ft in range(N_FT):
                    filler.append(
                        (lambda f=ft: project_qk(qh_halves[0], wq_sb, qh_sb, "q", nb, f), C_QK, key)
                    )
                for m in range(4):
                    filler.append(
                        (lambda x=m: project_v(vh_halves[0], nb, x), C_V, key)
                    )

            # block (·, 0) prerequisites run serially; everything later is
            # interleaved into the attention stream.  DMA issue order
            # interleaves weight halves with token halves so the first
            # k-proj matmul (needs wk half 0 + k-tokens half 0) starts
            # ~3us in.
            def w_rearr(w):
                if FP8:
                    return w.rearrange("(ct p) (two f) -> p ct two f", p=P, two=2)
                return w.rearrange("(ct p) f -> p ct f", p=P)

            def load_w(w_sb, w):
                wr = w_rearr(w)
                for h in range(2):
                    nc.sync.dma_start(
                        w_sb[:, 4 * h : 4 * h + 4], wr[:, 4 * h : 4 * h + 4]
                    )

            wkr = w_rearr(wk)
            kh0 = []
            for h in range(4):
                nc.sync.dma_start(
                    wk_sb[:, 2 * h : 2 * h + 2], wkr[:, 2 * h : 2 * h + 2]
                )
                if FP8:
                    sth = stA.tile([P, 2, 2, SB], IN_DT, tag=f"sk{h}", name=f"sk{h}")
                    nc.sync.dma_start(
                        sth,
                        kT.rearrange("(ct p) n2 -> p ct n2", p=P)[
                            :, 2 * h : 2 * h + 2, 0 : 2 * SB
                        ],
                    )
                else:
                    sth = stA.tile([P, 2, SB], IN_DT, tag=f"sk{h}", name=f"sk{h}")
                    nc.sync.dma_start(
                        sth,
                        kT[:, 0:SB].rearrange("(ct p) n -> p ct n", p=P)[
                            :, 2 * h : 2 * h + 2
                        ],
                    )
                kh0.append(sth)
            load_w(wq_sb, wq)
            qh0 = load_st(qT, 0, "sq")
            load_w(wv_sb, wv)
            vh0 = load_st(vT, 0, "sv")
            nc.sync.dma_start(
                wo_sb, wo.rearrange("(kt p) (dt q) -> p kt dt q", p=P, q=P)
            )
            if n_patterns:
                nc.sync.dma_start(mask_sb, maskt.rearrange("m p s -> p m s"))
            if has_bias:
                nc.sync.dma_start(
                    bias_sb["q"], bq.rearrange("(ft p) o -> p (ft o)", p=P)
                )
                nc.sync.dma_start(
                    bias_sb["k"], bk.rearrange("(ft p) o -> p (ft o)", p=P)
                )
            for ft in range(N_FT):
                project_qk(kh0, wk_sb, kh_sb, "k", 0, ft)
            for ft in range(N_FT):
                project_qk(qh0, wq_sb, qh_sb, "q", 0, ft)
            for m in range(4):
                project_v(vh0, 0, m)
            for nb in range(1, N_NB):
                push_block_units(nb)
            last = N_SB - 1
            for j in range(N_SB):
                attention_block(0, j)
                if j == last:
                    # kt=0 half of the last block's outproj runs as filler
                    # during the final attention block; kt=1 follows after
                    # (host sums the two partials).  Two nb2 units are held
                    # in reserve behind them (cost 99 blocks budget-drain)
                    # to keep PE hot through the final normalize chain.
                    for dg in range(N_CT // 2):
                        filler.append(
                            (lambda g=dg: outproj_nb(last, g, kts=(0,)), 1, None)
                        )
                    for g in reserve:
                        # the key is never force-flushed and the huge cost
                        # is never budget-drained -- these pop only in the
                        # final drain loop, after the last normalize is
                        # emitted, keeping PE hot through that chain
                        filler.append(
                            (
                                lambda x=g: outproj_nb(last - 1, x, tail=True),
                                10**9,
                                (999, 0),
                            )
                        )
                attention_block(1, j)
                if j == last:
                    for dg in range(N_CT // 2):
                        filler.append(
                            (
                                lambda g=dg: outproj_nb(
                                    last, g, kts=(1,), out_col0=NTOK_LOC,
                                    tail=True,
                                ),
                                1,
                                None,
                            )
                        )
                else:
                    dgs = range(N_CT // 2)
                    if j == last - 1:
                        dgs = range(N_CT // 2 - len(reserve))
                    for dg in dgs:
                        # higher cost spreads outproj into the
                        # (filler-starved) late attention rounds
                        filler.append((lambda x=j, g=dg: outproj_nb(x, g), C_OP, None))
            while filler:
                filler.popleft()[0]()

    nc.compile()
    return nc


_NC_CACHE = {}


def _get_nc(kind_key, kind, mix_idx, n_patterns, has_bias):
    key = (kind_key, n_patterns, has_bias)
    if key not in _NC_CACHE:
        _NC_CACHE[key] = _build_nc(kind, mix_idx, n_patterns, has_bias)
    return _NC_CACHE[key]


F8 = ml_dtypes.float8_e4m3


def _split8(x):
    """fp32 [D, N] -> [D, 2N] fp8e4 with (hi, lo) interleaved per
    512-token block: layout [D, nb, {hi, lo}, SB]."""
    d = x.shape[0]
    hi = x.astype(F8)
    lo = (x - hi.astype(np.float32)).astype(F8)
    a = np.stack(
        [hi.reshape(d, -1, SB), lo.reshape(d, -1, SB)], axis=2
    )  # [D, nb, 2, SB]
    return np.ascontiguousarray(a.reshape(d, -1))


def _splitw8(wT):
    """Weight [D, F] fp32, pre-scaled: -> [D, 2F] fp8 in (lo, hi) order."""
    ws = wT * W_SCALE
    hi = ws.astype(F8)
    lo = (ws - hi.astype(np.float32)).astype(F8)
    return np.ascontiguousarray(
        np.stack([lo, hi], axis=1).reshape(wT.shape[0], -1)
    )


def kernel(v, k, q, mask, Wq, bq, Wk, bk, Wv, bv, Wo, bo, trace=False):
    global LAST_RESULTS, LAST_EXEC_WALL
    from concourse.bass_utils import run_bass_kernel_spmd

    in_np = ml_dtypes.bfloat16

    def prep_T(x):  # [S, D] -> [D, S] in input dtype (or fp8 hi/lo pair)
        xT = np.ascontiguousarray(np.asarray(x, dtype=np.float32).T)
        if FP8:
            return _split8(xT)
        return xT.astype(in_np)

    kind, mix_idx, patterns = _mask_structure(np.asarray(mask, dtype=np.float32))
    maskt = (
        np.ascontiguousarray(np.stack(patterns)).astype(ml_dtypes.bfloat16)
        if patterns
        else None
    )

    has_bias = bool(np.any(np.asarray(bq)) or np.any(np.asarray(bk)))
    kind_key = str(kind) + str(mix_idx)
    nc = _get_nc(kind_key, kind, mix_idx, len(patterns), has_bias)

    q_np = np.asarray(q, np.float32)
    k_np = np.asarray(k, np.float32)
    v_np = np.asarray(v, np.float32)
    qT = [prep_T(q_np[b]) for b in range(BATCH)]
    kT = [prep_T(k_np[b]) for b in range(BATCH)]
    vT = [prep_T(v_np[b]) for b in range(BATCH)]

    in_maps = []
    for core in range(N_CORES):
        b = core // 4
        hg = core % 4
        fsl = slice(hg * FW, (hg + 1) * FW)
        def prep_w(W):
            wT = np.ascontiguousarray(np.asarray(W, np.float32)[fsl].T)
            if FP8:
                return _splitw8(wT)
            return wT.astype(in_np)

        wo_scale = W_SCALE if FP8 else 1.0
        m = {
            "qT": qT[b],
            "kT": kT[b],
            "vT": vT[b],
            "wqT": prep_w(Wq),
            "wkT": prep_w(Wk),
            "wvT": prep_w(Wv),
            "woT": np.ascontiguousarray(
                np.asarray(Wo, np.float32)[:, fsl].T / wo_scale
            ),
        }
        if maskt is not None:
            m["maskt"] = maskt
        if has_bias:
            # projections are scaled by W_SCALE in fp8 mode; scale the
            # biases to match (the exp scale folds it back out)
            m["bq"] = np.asarray(bq, np.float32)[fsl].reshape(FW, 1) * wo_scale
            m["bk"] = np.asarray(bk, np.float32)[fsl].reshape(FW, 1) * wo_scale
        in_maps.append(m)

    import time as _time

    _t0 = _time.time()
    res = run_bass_kernel_spmd(
        nc, in_maps, core_ids=list(range(N_CORES)), trace=trace
    )
    LAST_EXEC_WALL = _time.time() - _t0
    LAST_RESULTS = res

    out = np.zeros((BATCH, SEQ, D_MODEL), dtype=np.float32)
    last0 = (N_NB - 1) * SB
    for core in range(N_CORES):
        b = core // 4
        oT = res.results[core]["outT"]
        out[b] += oT[:, :NTOK_LOC].T
        # kt=1 partial of the last n-block lives in the extra columns
        out[b, last0 : last0 + SB] += oT[:, NTOK_LOC:].T
    # v-bias contributes the constant bv @ Wo.T (softmax rows sum to 1)
    out += (
        np.asarray(bo, np.float32)
        + np.asarray(bv, np.float32) @ np.asarray(Wo, np.float32).T
    )[None, None, :]
    return out
